# revision 1
# baseline (speedup 1.0000x reference)
"""Weighted 2D Gaussian KDE on 8 Trainium2 NeuronCores (Bass/Tile).

out[b,l] = sum_n w[n] * exp(-||x[b,l] - data[n]||^2 / sigma),  sigma = 3.

Algorithm (grid quadrature factorization, exponentially accurate):
  The per-dim Gaussian admits  exp(-(s-t)^2/sigma)
      = F * sum_j exp(-a (s-u_j)^2) * exp(-a (u_j-t)^2)
  over a uniform grid u_j with spacing h, where a = 2/sigma and
  F = h*sqrt(2a/pi); the quadrature error is ~2*exp(-pi^2/(2*a*h^2)).
  The 2D kernel separates per dim, so with grid matrices
      P_d[j,n] = exp(-a (u_j - data_n_d)^2),  Q_d[j,c] = exp(-a (u_j - x_c_d)^2)
  the KDE becomes out[c] = Q0[:,c]^T (F^2 * P0 diag(w) P1^T) Q1[:,c].
  This needs O((N+L)*G) exps instead of O(L*N), G = 64 grid nodes.

Sharding: locations (B*L = 131072) are split contiguously across the 8
cores (16384 each); data/weights are replicated; the small moment matrix
M is computed redundantly on every core (no collectives).
"""

import os
import numpy as np

import concourse.bass as bass
import concourse.tile as tile
from concourse import bacc
from concourse import mybir
from concourse import bass_utils

# ---- problem constants (hardcoded per spec) ----
B, L, D = 2, 65536, 2
NPTS = 16384
NCORES = 8
NLOC = B * L              # 131072 locations
NSH = NLOC // NCORES      # 16384 per core
SIGMA = 3.0
A = 2.0 / SIGMA           # per-dim quadrature gaussian coefficient
H = 0.75                  # grid spacing
PADG = 3.0                # grid extension beyond data/location range
NG = 64                   # grid size padded to 64 partitions
FQ = float(H * np.sqrt(2.0 * A / np.pi))  # per-dim quadrature factor
NSTRIP = 512
NSTRIPS_PTS = NPTS // NSTRIP   # 32
NSTRIPS_LOC = NSH // NSTRIP    # 32
OBLK = 4                       # out tiles buffered per DMA

F32 = mybir.dt.float32
F32R = mybir.dt.float32r
BF16 = mybir.dt.bfloat16
AF = mybir.ActivationFunctionType


def _r(ap):
    """Matmul operand dtype hook. Plain fp32 for now (4 cyc/row on PE);
    float32r (1 cyc/row) requires walrus-visible rounding of every producer."""
    return ap


def _build_core_program(nc: bass.Bass):
    xsh = nc.dram_tensor("xsh", [NSH, D], F32, kind="ExternalInput").ap()
    dat = nc.dram_tensor("dat", [NPTS, D], F32, kind="ExternalInput").ap()
    wts = nc.dram_tensor("wts", [NPTS], F32, kind="ExternalInput").ap()
    # host-computed grid rows:
    #   uga = [1; 2a*u; -a*u^2]   (stage A rhs)
    #   ugb = [-a*1; 2a*u; -a*u^2] (stage B lhsT)
    uga = nc.dram_tensor("uga", [2, NG], F32, kind="ExternalInput").ap()
    ugb = nc.dram_tensor("ugb", [2, NG], F32, kind="ExternalInput").ap()
    ubias = nc.dram_tensor("ubias", [NG, 1], F32, kind="ExternalInput").ap()
    out = nc.dram_tensor("out", [NSH], F32, kind="ExternalOutput").ap()

    with tile.TileContext(nc) as tc:
        with (
            tc.tile_pool(name="const", bufs=1) as const,
            tc.tile_pool(name="aug", bufs=1) as augp,
            tc.tile_pool(name="small", bufs=1) as small,
            tc.tile_pool(name="sbP", bufs=3) as sbP,
            tc.tile_pool(name="sbPT", bufs=4) as sbPT,
            tc.tile_pool(name="sbQ", bufs=3) as sbQ,
            tc.tile_pool(name="sbO", bufs=2) as sbO,
            tc.tile_pool(name="psum", bufs=2, space="PSUM") as psum,
        ):
            uga_sb = const.tile([2, NG], F32)
            nc.sync.dma_start(out=uga_sb, in_=uga)
            ugb_sb = const.tile([2, NG], F32)
            nc.sync.dma_start(out=ugb_sb, in_=ugb)
            ubias_sb = const.tile([NG, 1], F32)
            nc.sync.dma_start(out=ubias_sb, in_=ubias)
            ones_col = const.tile([NG, 1], F32)
            nc.vector.memset(ones_col, 1.0)
            m2sb = const.tile([NG, NG], F32)

            def aug_compute(src_ap, n_elems, dim, with_lnw, nega, tag):
                """Chunked [128, f] compute of v and s*v^2 (+ ln w)."""
                p = 128
                f = n_elems // p
                ct = small.tile([p, f], F32, tag=f"ct{tag}")
                nc.sync.dma_start(
                    out=ct, in_=src_ap.rearrange("(p f) d -> d p f", p=p)[dim]
                )
                sq = small.tile([p, f], F32, tag=f"sq{tag}")
                nc.vector.tensor_mul(sq, ct, ct)
                if nega:
                    nc.vector.tensor_scalar_mul(sq, sq, -A)
                if with_lnw:
                    wt = small.tile([p, f], F32, tag="wt")
                    nc.sync.dma_start(
                        out=wt, in_=wts.rearrange("(p f) -> p f", p=p)
                    )
                    lnw = small.tile([p, f], F32, tag="lnw")
                    nc.scalar.activation(lnw, wt, AF.Ln)
                    nc.vector.tensor_scalar_max(lnw, lnw, -1e30)
                    nc.vector.tensor_add(sq, sq, lnw)
                return sq, ct

            def aug_assemble(aug, sq, ct, n_elems):
                """Scatter chunked rows into [2, n] aug via SBUF->SBUF DMA.

                Call only right after a strict barrier: these DMAs must
                carry no sync waits (walrus wait-table limits).
                """
                p = 128
                a3 = aug.rearrange("r (p f) -> r p f", p=p)
                nc.sync.dma_start(out=a3[0:1], in_=sq)
                nc.sync.dma_start(out=a3[1:2], in_=ct)

            # ---------------- stage A: moment matrix M ----------------
            # augA rows: [-a*d^2 (+lnw); d; 1]; arg = augA^T-contraction with
            # uga -> arg[j, n] per strip; P = exp(arg) in bf16; DMA-transpose
            # to [n, j] chunks; M2[k, j] += P1T^T P0T accumulated in PSUM.
            augA0 = augp.tile([2, NPTS], F32, tag="aug0")
            augA1 = augp.tile([2, NPTS], F32, tag="aug1")
            # stage A needs -a*d^2 on row0 for BOTH dims (uga row0 is 1)
            sqa0, cta0 = aug_compute(dat, NPTS, 0, True, True, "a0")
            sqa1, cta1 = aug_compute(dat, NPTS, 1, False, True, "a1")
            sqb0, ctb0 = aug_compute(xsh, NSH, 0, False, False, "b0")
            sqb1, ctb1 = aug_compute(xsh, NSH, 1, False, False, "b1")
            tc.strict_bb_all_engine_barrier()
            aug_assemble(augA0, sqa0, cta0, NPTS)
            aug_assemble(augA1, sqa1, cta1, NPTS)
            tc.strict_bb_all_engine_barrier()

            m2ps = psum.tile([NG, NG], F32, tag="M2", bufs=1)
            for s in range(NSTRIPS_PTS):
                sl = slice(s * NSTRIP, (s + 1) * NSTRIP)
                pst = []
                for dim, aug in ((0, augA0), (1, augA1)):
                    argp = psum.tile([NG, NSTRIP], F32, tag=f"arg{dim}")
                    nc.tensor.matmul(
                        argp, lhsT=_r(uga_sb), rhs=_r(aug[:, sl]),
                        start=True, stop=True,
                    )
                    p_bf = sbP.tile([NG, NSTRIP], BF16, tag=f"P{dim}")
                    nc.scalar.activation(p_bf, argp, AF.Exp, bias=ubias_sb)
                    pst.append(p_bf)
                for i in range(NSTRIP // 128):
                    csl = slice(i * 128, (i + 1) * 128)
                    pt0 = sbPT.tile([128, NG], BF16, tag="PT0")
                    nc.sync.dma_start(out=pt0, in_=pst[0][:, csl], transpose=True)
                    pt1 = sbPT.tile([128, NG], BF16, tag="PT1")
                    nc.scalar.dma_start(out=pt1, in_=pst[1][:, csl], transpose=True)
                    nc.tensor.matmul(
                        m2ps, lhsT=pt1, rhs=pt0,
                        start=(s == 0 and i == 0),
                        stop=(s == NSTRIPS_PTS - 1 and i == NSTRIP // 128 - 1),
                        skip_group_check=True,
                    )
            # fold the 2D quadrature factor while copying PSUM -> SBUF
            nc.scalar.mul(m2sb, m2ps, FQ * FQ)

            # ---------------- stage B: per-location evaluation ----------------
            tc.strict_bb_all_engine_barrier()
            augB0 = augp.tile([2, NSH], F32, tag="aug0")
            augB1 = augp.tile([2, NSH], F32, tag="aug1")
            aug_assemble(augB0, sqb0, ctb0, NSH)
            aug_assemble(augB1, sqb1, ctb1, NSH)
            tc.strict_bb_all_engine_barrier()

            for t in range(NSTRIPS_LOC):
                sl = slice(t * NSTRIP, (t + 1) * NSTRIP)
                qt = []
                for dim, aug in ((0, augB0), (1, augB1)):
                    argp = psum.tile([NG, NSTRIP], F32, tag=f"arg{dim}")
                    nc.tensor.matmul(
                        argp, lhsT=_r(ugb_sb), rhs=_r(aug[:, sl]),
                        start=True, stop=True,
                    )
                    q = sbQ.tile([NG, NSTRIP], F32, tag=f"Q{dim}")
                    nc.scalar.activation(q, argp, AF.Exp, bias=ubias_sb)
                    qt.append(q)
                tps = psum.tile([NG, NSTRIP], F32, tag="T")
                nc.tensor.matmul(
                    tps, lhsT=_r(m2sb), rhs=_r(qt[1]), start=True, stop=True
                )
                r = sbQ.tile([NG, NSTRIP], F32, tag="R")
                nc.vector.tensor_mul(r, qt[0], tps)
                ops = psum.tile([1, NSTRIP], F32, tag="o", bufs=1)
                nc.tensor.matmul(
                    ops, lhsT=_r(ones_col), rhs=_r(r), start=True, stop=True
                )
                if t % OBLK == 0:
                    ob = sbO.tile([1, OBLK * NSTRIP], F32, tag="ob")
                nc.vector.tensor_copy(
                    ob[:, (t % OBLK) * NSTRIP : (t % OBLK + 1) * NSTRIP], ops
                )
                if t % OBLK == OBLK - 1:
                    t0 = t - (OBLK - 1)
                    nc.sync.dma_start(
                        out=out.rearrange("(o n) -> o n", o=1)[
                            :, t0 * NSTRIP : (t0 + OBLK) * NSTRIP
                        ],
                        in_=ob,
                    )
    return nc


_CACHE = {}
LAST_RESULTS = None


def _get_nc():
    if "nc" not in _CACHE:
        nc = bacc.Bacc("TRN2", target_bir_lowering=False, debug=False)
        _build_core_program(nc)
        nc.compile()
        _CACHE["nc"] = nc
    return _CACHE["nc"]


def _host_grid(x, data):
    """Host-side prep: the 64-node grid rows (tiny, value-dependent)."""
    lo = float(min(x.min(), data.min())) - PADG
    hi = float(max(x.max(), data.max())) + PADG
    ng = int(np.ceil((hi - lo) / H)) + 1
    assert ng <= NG, f"grid {ng} exceeds padded size {NG}"
    u = np.empty(NG, np.float64)
    u[:ng] = lo + np.arange(ng) * H
    u[ng:] = -1e4  # far away: padded nodes contribute exactly 0
    uga = np.stack([np.ones(NG), 2.0 * A * u]).astype(np.float32)
    ugb = np.stack([-A * np.ones(NG), 2.0 * A * u]).astype(np.float32)
    ubias = (-A * u * u).astype(np.float32)[:, None]
    return uga, ugb, ubias


def kernel(x, data, weights):
    global LAST_RESULTS
    x = np.ascontiguousarray(x, dtype=np.float32)
    data = np.ascontiguousarray(data, dtype=np.float32)
    weights = np.ascontiguousarray(weights, dtype=np.float32)
    assert x.shape == (B, L, D) and data.shape == (NPTS, D)

    uga, ugb, ubias = _host_grid(x, data)
    xf = x.reshape(NLOC, D)
    in_maps = []
    for c in range(NCORES):
        in_maps.append({
            "xsh": np.ascontiguousarray(xf[c * NSH : (c + 1) * NSH]),
            "dat": data,
            "wts": weights,
            "uga": uga,
            "ugb": ugb,
            "ubias": ubias,
        })

    nc = _get_nc()
    res = bass_utils.run_bass_kernel_spmd(
        nc, in_maps, core_ids=list(range(NCORES)),
        trace=bool(os.environ.get("BASS_TRACE")),
    )
    LAST_RESULTS = res
    out = np.concatenate([res.results[c]["out"] for c in range(NCORES)])
    return out.reshape(B, L)



# revision 2
# speedup vs baseline: 5.5354x; 5.5354x over previous
"""Weighted 2D Gaussian KDE on 8 Trainium2 NeuronCores (Bass/Tile).

out[b,l] = sum_n w[n] * exp(-||x[b,l] - data[n]||^2 / sigma),  sigma = 3.

Grid-quadrature factorization (exponentially accurate):
  exp(-(s-t)^2/sigma) = F * sum_j exp(-a(s-u_j)^2) * exp(-a(u_j-t)^2)
  over a uniform grid u_j (spacing h, a = 2/sigma, F = h*sqrt(2a/pi)).
  The 2D kernel separates per dim, so with the 64x64 moment matrix
  M = F^2 * P1 diag(w) P0^T the KDE is out[c] = q0[:,c]^T M^T q1[:,c].

Device pipeline (per core, locations sharded 16384/core):
  stage A (points, transposed): per 128-point chunk, one K=12 bf16 matmul
  produces exp-args for both dims ([128 pts, 128 grid]); ScalarE exps them;
  a K=128 matmul accumulates M into PSUM. No DMA transposes.
  stage B (locations): per 512-loc strip, one K=10 bf16 matmul + ScalarE exp
  gives Q [128, 512] (partitions 0-63 dim1, 64-127 dim0); T = M^T q1 on PE;
  r = q0*T on DVE; a ones-matmul accumulates strip sums into one [32, 512]
  PSUM tile that is the final output layout.

All matmul operands are bf16; fp32 accuracy of the exp arguments is kept by
hi/lo bf16 splits of every product term (host-precomputed aug tensors), so
the PE streams at 1 col/cycle instead of fp32's LOW_HIGH half rate.
"""

import os
import numpy as np
import ml_dtypes

import concourse.bass as bass
import concourse.tile as tile
from concourse import bacc
from concourse import mybir
from concourse import bass_utils

# ---- problem constants (hardcoded per spec) ----
B, L, D = 2, 65536, 2
NPTS = 16384
NCORES = 8
NLOC = B * L
NSH = NLOC // NCORES      # 16384 locations per core
SIGMA = 3.0
A = 2.0 / SIGMA
H = 0.75
PADG = 3.0
NG = 64                   # grid nodes per dim (padded)
FQ = float(H * np.sqrt(2.0 * A / np.pi))
NSTRIP = 512
NSTRIPS = NSH // NSTRIP   # 32
NCHUNK = NPTS // 128      # 128 point chunks
KA = 12                   # stage A contraction rows
KB = 10                   # stage B contraction rows

F32 = mybir.dt.float32
BF16 = mybir.dt.bfloat16
BF = ml_dtypes.bfloat16
AF = mybir.ActivationFunctionType


def _build_core_program(nc: bass.Bass):
    augA = nc.dram_tensor("augA", [KA, NPTS], BF16, kind="ExternalInput").ap()
    uwa = nc.dram_tensor("uwa", [KA, 128], BF16, kind="ExternalInput").ap()
    augB = nc.dram_tensor("augB", [KB, NSH], BF16, kind="ExternalInput").ap()
    uwb = nc.dram_tensor("uwb", [KB, 128], BF16, kind="ExternalInput").ap()
    ubias = nc.dram_tensor("ubias", [128, 1], F32, kind="ExternalInput").ap()
    onesw = nc.dram_tensor("onesw", [64, 32 * 32], BF16, kind="ExternalInput").ap()
    out = nc.dram_tensor("out", [NSH], F32, kind="ExternalOutput").ap()

    with tile.TileContext(nc) as tc:
        with (
            tc.tile_pool(name="const", bufs=1) as const,
            tc.tile_pool(name="sbP", bufs=3) as sbP,
            tc.tile_pool(name="sbQ", bufs=3) as sbQ,
            tc.tile_pool(name="sbR", bufs=2) as sbR,
            tc.tile_pool(name="sbO", bufs=1) as sbO,
            tc.tile_pool(name="psA", bufs=2, space="PSUM") as psA,
            tc.tile_pool(name="psB", bufs=2, space="PSUM") as psB,
            tc.tile_pool(name="psT", bufs=2, space="PSUM") as psT,
            tc.tile_pool(name="psS", bufs=1, space="PSUM") as psS,
        ):
            augA_sb = const.tile([KA, NPTS], BF16)
            nc.sync.dma_start(out=augA_sb, in_=augA)
            uwa_sb = const.tile([KA, 128], BF16)
            nc.sync.dma_start(out=uwa_sb, in_=uwa)
            augB_sb = const.tile([KB, NSH], BF16)
            nc.sync.dma_start(out=augB_sb, in_=augB)
            uwb_sb = const.tile([KB, 128], BF16)
            nc.sync.dma_start(out=uwb_sb, in_=uwb)
            ubias_sb = const.tile([128, 1], F32)
            nc.sync.dma_start(out=ubias_sb, in_=ubias)
            onesw_sb = const.tile([128, 32 * 32], BF16)
            nc.sync.dma_start(out=onesw_sb[64:128, :], in_=onesw)
            m2t = const.tile([64, 64], BF16)

            m2ps = psS.tile([64, 64], F32, tag="m2", bufs=1)
            outps = psS.tile([32, NSTRIP], F32, tag="out", bufs=1)

            # ---------------- stage A: moment matrix M ----------------
            # software-pipelined: arg matmuls for group g+1 are emitted before
            # the exp/accum of group g so the PE never waits on ScalarE.
            NGRP = NCHUNK // 4  # 32 groups of 4 chunks
            argA = [None] * NGRP
            p4 = [None] * NGRP

            def emit_argA(g):
                ps = psA.tile([128, 512], F32, tag="argA")
                for s in range(4):
                    c = g * 4 + s
                    nc.tensor.matmul(
                        ps[:, s * 128 : (s + 1) * 128],
                        lhsT=augA_sb[:, c * 128 : (c + 1) * 128],
                        rhs=uwa_sb,
                        start=True, stop=True, skip_group_check=True,
                    )
                argA[g] = ps

            def emit_expA(g):
                p = sbP.tile([128, 512], BF16, tag="P")
                nc.scalar.activation(p, argA[g], AF.Exp)
                p4[g] = p

            def emit_accumA(g):
                p = p4[g]
                for s in range(4):
                    c = g * 4 + s
                    nc.tensor.matmul(
                        m2ps,
                        lhsT=p[:, s * 128 : s * 128 + 64],
                        rhs=p[:, s * 128 + 64 : s * 128 + 128],
                        start=(c == 0), stop=(c == NCHUNK - 1),
                        skip_group_check=True,
                    )

            for g in range(NGRP + 2):
                if g < NGRP:
                    emit_argA(g)
                    emit_expA(g)
                if g >= 2:
                    emit_accumA(g - 2)
            # fold the 2D quadrature factor while copying PSUM -> SBUF bf16
            nc.scalar.mul(m2t, m2ps, FQ * FQ)

            # ---------------- stage B: per-location evaluation ----------------
            qt = [None] * NSTRIPS

            def emit_argB(t):
                ps = psB.tile([128, NSTRIP], F32, tag="argB")
                nc.tensor.matmul(
                    ps, lhsT=uwb_sb, rhs=augB_sb[:, t * NSTRIP : (t + 1) * NSTRIP],
                    start=True, stop=True,
                )
                q = sbQ.tile([128, NSTRIP], BF16, tag="Q")
                nc.scalar.activation(q, ps, AF.Exp, bias=ubias_sb)
                qt[t] = q

            def emit_tail(t):
                q = qt[t]
                tp = psT.tile([128, NSTRIP], F32, tag="T")
                nc.tensor.matmul(
                    tp[64:128, :], lhsT=m2t, rhs=q[0:64, :],
                    start=True, stop=True,
                )
                r = sbR.tile([128, NSTRIP], BF16, tag="r")
                nc.vector.tensor_mul(r[64:128, :], q[64:128, :], tp[64:128, :])
                nc.tensor.matmul(
                    outps,
                    lhsT=onesw_sb[64:128, t * 32 : (t + 1) * 32],
                    rhs=r[64:128, :],
                    start=(t == 0), stop=(t == NSTRIPS - 1),
                    skip_group_check=True,
                )

            for t in range(NSTRIPS + 2):
                if t < NSTRIPS:
                    emit_argB(t)
                if t >= 2:
                    emit_tail(t - 2)

            outsb = sbO.tile([32, NSTRIP], F32)
            nc.vector.tensor_copy(outsb, outps)
            nc.sync.dma_start(
                out=out.rearrange("(p f) -> p f", p=32), in_=outsb
            )
    return nc


_CACHE = {}
LAST_RESULTS = None


def _get_nc():
    if "nc" not in _CACHE:
        nc = bacc.Bacc("TRN2", target_bir_lowering=False, debug=False)
        _build_core_program(nc)
        nc.compile()
        _CACHE["nc"] = nc
    return _CACHE["nc"]


def _bf(v):
    return np.asarray(v, dtype=BF)


def _split2(v):
    """fp64 -> (hi, lo) bf16 pair with hi+lo ~ fp32(v)."""
    hi = _bf(v)
    lo = _bf(np.asarray(v, np.float32) - hi.astype(np.float32))
    return hi, lo


def _host_prep(x, data, weights):
    lo = float(min(x.min(), data.min())) - PADG
    hi = float(max(x.max(), data.max())) + PADG
    ng = int(np.ceil((hi - lo) / H)) + 1
    assert ng <= NG, f"grid {ng} exceeds padded size {NG}"
    u = np.full(NG, -1000.0)
    u[:ng] = lo + np.arange(ng) * H

    w1, w1f = _split2(2.0 * A * u)
    v1, v1f = _split2(-A * u * u)
    # jcat layout: cols/partitions 0-63 = dim1 (q1 side), 64-127 = dim0
    m1 = np.zeros(128); m1[:64] = 1.0
    m0 = np.zeros(128); m0[64:] = 1.0
    w1c = np.tile(w1.astype(np.float64), 2)
    w1fc = np.tile(w1f.astype(np.float64), 2)
    v1c = np.tile(v1.astype(np.float64), 2)
    v1fc = np.tile(v1f.astype(np.float64), 2)
    ubias = np.tile(-A * u * u, 2).astype(np.float32)[:, None]

    # stage A point-side rows + grid-side moving operand
    d0 = data[:, 0].astype(np.float64)
    d1 = data[:, 1].astype(np.float64)
    lnw = np.maximum(
        np.log(np.maximum(weights.astype(np.float64), 1e-300)), -200.0
    )
    d0c, d0f = _split2(d0)
    d1c, d1f = _split2(d1)
    e0h, e0l = _split2(-A * d0 * d0 + lnw)
    e1h, e1l = _split2(-A * d1 * d1)
    one = np.ones(NPTS)
    augA = _bf(np.stack(
        [d0c, d0f, d0c, e0h, e0l, d1c, d1f, d1c, e1h, e1l, one, one]
    ))
    uwa = _bf(np.stack([
        w1c * m0, w1c * m0, w1fc * m0, m0, m0,
        w1c * m1, w1c * m1, w1fc * m1, m1, m1,
        v1c, v1fc,
    ]))
    uwb = _bf(np.stack([
        w1c * m0, w1c * m0, w1fc * m0, m0, m0,
        w1c * m1, w1c * m1, w1fc * m1, m1, m1,
    ]))
    onesw = np.zeros((64, 32 * 32), np.float64)
    for t in range(32):
        onesw[:, 32 * t + t] = 1.0
    return augA, uwa, uwb, _bf(onesw), ubias, (m0, m1)


def _prep_augB(xsh):
    x0 = xsh[:, 0].astype(np.float64)
    x1 = xsh[:, 1].astype(np.float64)
    xc0, xf0 = _split2(x0)
    xc1, xf1 = _split2(x1)
    x2h0, x2l0 = _split2(-A * x0 * x0)
    x2h1, x2l1 = _split2(-A * x1 * x1)
    return _bf(np.stack(
        [xc0, xf0, xc0, x2h0, x2l0, xc1, xf1, xc1, x2h1, x2l1]
    ))


def kernel(x, data, weights):
    global LAST_RESULTS
    x = np.ascontiguousarray(x, dtype=np.float32)
    data = np.ascontiguousarray(data, dtype=np.float32)
    weights = np.ascontiguousarray(weights, dtype=np.float32)
    assert x.shape == (B, L, D) and data.shape == (NPTS, D)

    augA, uwa, uwb, onesw, ubias, _ = _host_prep(x, data, weights)
    xf = x.reshape(NLOC, D)
    in_maps = []
    for c in range(NCORES):
        in_maps.append({
            "augA": augA,
            "uwa": uwa,
            "augB": _prep_augB(xf[c * NSH : (c + 1) * NSH]),
            "uwb": uwb,
            "ubias": ubias,
            "onesw": onesw,
        })

    nc = _get_nc()
    res = bass_utils.run_bass_kernel_spmd(
        nc, in_maps, core_ids=list(range(NCORES)),
        trace=bool(os.environ.get("BASS_TRACE")),
    )
    LAST_RESULTS = res
    out = np.concatenate([res.results[c]["out"] for c in range(NCORES)])
    return out.reshape(B, L)
